# revision 1
# baseline (speedup 1.0000x reference)
"""MoE expert routing kernel for Trainium2 (8 NeuronCores).

Problem: out[b] = x[b] @ weight[index[b]] + bias[index[b]]
  x: (4096, 256) f32, index: (4096,) i32 in [0,32),
  weight: (32, 256, 256) f32, bias: (32, 256) f32.

Strategy (expert-parallel, host-side routing):
  - Host sorts tokens by expert (stable argsort) and pads each expert's
    token group to a fixed capacity C (multiple of 128).
  - Core c owns experts [4c, 4c+4): it receives the padded, transposed
    token block xT [256, 4*C], its 4 expert weights [4, 256, 256] and
    biases. All cores run the same (SPMD) program.
  - On device, per expert: out[t, o] = sum_k xT[k, t] * W[k, o] + b[o]
    as PE matmuls (lhsT = xT tile stationary, rhs = W half moving),
    K split 128+128, bias added via a K=1 matmul with a ones row.
  - Host unpermutes the gathered per-core outputs.
"""

import numpy as np

B, E, DIN, DOUT = 4096, 32, 256, 256
NCORES = 8
EPC = E // NCORES  # experts per core

# Set by test harness to capture a perfetto trace / exec time.
TRACE = False
LAST_RESULT = None

_PROGRAM_CACHE = {}


def _build_program(C):
    """Build the SPMD Bass program for per-expert capacity C (mult of 128)."""
    import concourse.bass as bass
    import concourse.mybir as mybir
    import concourse.tile as tile
    from concourse import bacc

    f32 = mybir.dt.float32
    f32r = mybir.dt.float32r

    TC = EPC * C          # tokens (padded) per core
    GE = C // 128         # 128-token tiles per expert
    KH = DIN // 128       # contraction halves

    nc = bacc.Bacc("TRN2", target_bir_lowering=False, debug=False,
                   enable_asserts=False)

    xT_d = nc.dram_tensor("xT", [DIN, TC], f32r, kind="ExternalInput")
    w_d = nc.dram_tensor("w", [EPC, DIN, DOUT], f32r, kind="ExternalInput")
    # bias flattened with a trailing block of 128 ones (for the bias matmul)
    bc_d = nc.dram_tensor("bcat", [1, EPC * DOUT + 128], f32r,
                          kind="ExternalInput")
    out_d = nc.dram_tensor("out", [TC, DOUT], f32, kind="ExternalOutput")

    xT_v = xT_d.ap().rearrange("(h p) t -> p h t", p=128)      # [128, KH, TC]
    w_v = w_d.ap().rearrange("e (h p) o -> e p h o", p=128)    # [EPC,128,KH,O]
    out_v = out_d.ap().rearrange("(g p) o -> p g o", p=128)    # [128, G, O]

    with tile.TileContext(nc) as tc:
        with (
            tc.tile_pool(name="const", bufs=1) as cpool,
            tc.tile_pool(name="xin", bufs=2) as xpool,
            tc.tile_pool(name="win", bufs=2) as wpool,
            tc.tile_pool(name="oout", bufs=2) as opool,
            tc.tile_pool(name="psum", bufs=4, space=bass.MemorySpace.PSUM)
                as ppool,
        ):
            bc = cpool.tile([1, EPC * DOUT + 128], f32r)
            nc.sync.dma_start(bc[:], bc_d.ap())
            ones = bc[0:1, EPC * DOUT:EPC * DOUT + 128]

            for e in range(EPC):
                xe = xpool.tile([128, KH, C], f32r)
                nc.sync.dma_start(xe[:], xT_v[:, :, e * C:(e + 1) * C])
                we = wpool.tile([128, KH, DOUT], f32r)
                nc.sync.dma_start(we[:], w_v[e])

                osb = opool.tile([128, GE, DOUT], f32)
                for tt in range(GE):
                    ps = ppool.tile([128, DOUT], f32)
                    for h in range(KH):
                        nc.tensor.matmul(
                            ps[:],
                            xe[:, h, tt * 128:(tt + 1) * 128],
                            we[:, h, :],
                            start=(h == 0), stop=False,
                        )
                    nc.tensor.matmul(
                        ps[:],
                        ones,
                        bc[0:1, e * DOUT:(e + 1) * DOUT],
                        start=False, stop=True,
                    )
                    nc.vector.tensor_copy(osb[:, tt, :], ps[:])
                nc.sync.dma_start(out_v[:, e * GE:(e + 1) * GE, :], osb[:])

    nc.compile()
    return nc


def _route(index):
    """Host-side routing: stable sort of tokens by expert + capacity."""
    order = np.argsort(index, kind="stable")
    counts = np.bincount(index, minlength=E)
    offs = np.zeros(E + 1, np.int64)
    offs[1:] = np.cumsum(counts)
    C = max(256, int(-(-int(counts.max()) // 128) * 128))
    return order, counts, offs, C


def kernel(x, index, weight, bias):
    from concourse.bass_utils import run_bass_kernel_spmd

    global LAST_RESULT

    x = np.ascontiguousarray(np.asarray(x, np.float32))
    index = np.asarray(index, np.int32)
    weight = np.ascontiguousarray(np.asarray(weight, np.float32))
    bias = np.ascontiguousarray(np.asarray(bias, np.float32))

    order, counts, offs, C = _route(index)
    TC = EPC * C

    if C not in _PROGRAM_CACHE:
        _PROGRAM_CACHE[C] = _build_program(C)
    nc = _PROGRAM_CACHE[C]

    ones128 = np.ones(128, np.float32)
    in_maps = []
    for c in range(NCORES):
        xT = np.zeros((DIN, TC), np.float32)
        for sl in range(EPC):
            e = c * EPC + sl
            toks = order[offs[e]:offs[e + 1]]
            xT[:, sl * C:sl * C + len(toks)] = x[toks].T
        bcat = np.concatenate(
            [bias[c * EPC:(c + 1) * EPC].reshape(-1), ones128])[None, :]
        in_maps.append({
            "xT": np.ascontiguousarray(xT),
            "w": np.ascontiguousarray(weight[c * EPC:(c + 1) * EPC]),
            "bcat": np.ascontiguousarray(bcat),
        })

    kwargs = {}
    if TRACE:
        kwargs = dict(trace=True, trace_cores=list(range(NCORES)))
    res = run_bass_kernel_spmd(nc, in_maps, core_ids=list(range(NCORES)),
                               **kwargs)
    LAST_RESULT = res

    out = np.empty((B, DOUT), np.float32)
    for c in range(NCORES):
        oc = res.results[c]["out"]
        for sl in range(EPC):
            e = c * EPC + sl
            toks = order[offs[e]:offs[e + 1]]
            out[toks] = oc[sl * C:sl * C + len(toks)]
    return out


# revision 2
# speedup vs baseline: 1.0273x; 1.0273x over previous
"""MoE expert-routing kernel for Trainium2 (8 NeuronCores).

out[b] = x[b] @ weight[index[b]] + bias[index[b]]

Strategy: expert-parallel sharding (4 experts per core), host-side token
routing (stable argsort by expert, padded to capacity C per expert), fp16 in/out, C=mult-of-64 capacity, fused per-expert
blocks on alternating HWDGE rings, bias fused into the DVE PSUM->SBUF add.

Host-packed fp16 block per expert:
  blk[e] = [w_h0 (256) | w_h1 (256) | bias_bcast (256) | xT_h0 (C) | xT_h1 (C)]
Output is fp16 on device, widened to fp32 on host.
"""

import numpy as np

B, E, DIN, DOUT = 4096, 32, 256, 256
NCORES = 8
EPC = E // NCORES

TRACE = False
LAST_RESULT = None

_PROGRAM_CACHE = {}


def _build_program(C):
    import concourse.bass as bass
    import concourse.mybir as mybir
    import concourse.tile as tile
    from concourse import bacc

    f32 = mybir.dt.float32
    f16 = mybir.dt.float16

    TC = EPC * C
    W = 3 * DOUT + 2 * C
    xoff = 3 * DOUT
    NT = -(-C // 128)

    nc = bacc.Bacc("TRN2", target_bir_lowering=False, debug=False,
                   enable_asserts=False)

    blk_d = nc.dram_tensor("blk", [EPC, 128, W], f16, kind="ExternalInput")
    out_d = nc.dram_tensor("out", [TC, DOUT], f16, kind="ExternalOutput")

    with tile.TileContext(nc) as tc:
        with (
            tc.tile_pool(name="bin", bufs=4) as bpool,
            tc.tile_pool(name="oout", bufs=4) as opool,
            tc.tile_pool(name="psum", bufs=6, space=bass.MemorySpace.PSUM)
                as ppool,
        ):
            blks = []
            for e in range(EPC):
                blk = bpool.tile([128, W], f16)
                eng = nc.sync if e % 2 == 0 else nc.scalar
                eng.dma_start(blk[:], blk_d.ap()[e])
                blks.append(blk)

            for e in range(EPC):
                blk = blks[e]
                for tt in range(NT):
                    m = min(128, C - tt * 128)
                    ps = ppool.tile([128, DOUT], f32)
                    for h in range(2):
                        nc.tensor.matmul(
                            ps[:m, :],
                            blk[:, xoff + h * C + tt * 128:
                                xoff + h * C + tt * 128 + m],
                            blk[:, h * DOUT:(h + 1) * DOUT],
                            start=(h == 0), stop=(h == 1),
                        )
                    ot = opool.tile([128, DOUT], f16)
                    nc.vector.tensor_add(ot[:m, :], ps[:m, :],
                                         blk[:m, 2 * DOUT:3 * DOUT])
                    r0 = e * C + tt * 128
                    eng = nc.sync if (e * NT + tt) % 2 == 0 else nc.scalar
                    eng.dma_start(out_d.ap()[r0:r0 + m, :], ot[:m, :])

    nc.compile()
    return nc


def _route(index):
    order = np.argsort(index, kind="stable")
    counts = np.bincount(index, minlength=E)
    offs = np.zeros(E + 1, np.int64)
    offs[1:] = np.cumsum(counts)
    C = max(64, int(-(-int(counts.max()) // 64) * 64))
    return order, counts, offs, C


def _pack_core(x16, w16, b16, order, offs, C, c):
    W = 3 * DOUT + 2 * C
    xoff = 3 * DOUT
    blk = np.zeros((EPC, 128, W), np.float16)
    for sl in range(EPC):
        e = c * EPC + sl
        toks = order[offs[e]:offs[e + 1]]
        xT = x16[toks].T
        blk[sl, :, 0:DOUT] = w16[e, 0:128, :]
        blk[sl, :, DOUT:2 * DOUT] = w16[e, 128:256, :]
        blk[sl, :, 2 * DOUT:3 * DOUT] = b16[e][None, :]
        blk[sl, :, xoff:xoff + xT.shape[1]] = xT[0:128]
        blk[sl, :, xoff + C:xoff + C + xT.shape[1]] = xT[128:256]
    return np.ascontiguousarray(blk)


def kernel(x, index, weight, bias):
    from concourse.bass_utils import run_bass_kernel_spmd

    global LAST_RESULT

    x = np.asarray(x, np.float32)
    index = np.asarray(index, np.int32)
    weight = np.asarray(weight, np.float32)
    bias = np.asarray(bias, np.float32)

    order, counts, offs, C = _route(index)

    if C not in _PROGRAM_CACHE:
        _PROGRAM_CACHE[C] = _build_program(C)
    nc = _PROGRAM_CACHE[C]

    x16 = x.astype(np.float16)
    w16 = weight.astype(np.float16)
    b16 = bias.astype(np.float16)
    in_maps = [{"blk": _pack_core(x16, w16, b16, order, offs, C, c)}
               for c in range(NCORES)]

    kwargs = {}
    if TRACE:
        kwargs = dict(trace=True, trace_cores=list(range(NCORES)))
    res = run_bass_kernel_spmd(nc, in_maps, core_ids=list(range(NCORES)),
                               **kwargs)
    LAST_RESULT = res

    out = np.empty((B, DOUT), np.float32)
    for c in range(NCORES):
        oc = res.results[c]["out"].astype(np.float32)
        for sl in range(EPC):
            e = c * EPC + sl
            toks = order[offs[e]:offs[e + 1]]
            out[toks] = oc[sl * C:sl * C + len(toks)]
    return out


# revision 3
# speedup vs baseline: 1.0929x; 1.0638x over previous
"""MoE expert-routing kernel for Trainium2 (8 NeuronCores).

out[b] = x[b] @ weight[index[b]] + bias[index[b]]

Expert-parallel sharding (4 experts/core), host-side token routing
(stable argsort, capacity C per expert), fp16 operands/output with fp32
PSUM accumulation. Transposed compute layout — weights stationary,
tokens moving:

out^T[o, t] = sum_i W[i, o] * xT[i, t] + b[o], per expert, computed as
2 o-half PSUM groups x 2 K-half matmuls with N = C tokens (C=192 < 256),
25% fewer streamed PE rows than the token-stationary layout. Bias is a
per-partition column -> DVE tensor_scalar_add during the PSUM->SBUF move.

Host-packed fp16 block per expert ([128, 4*128 + 2 + 2C]):
  blk[e] = [w(k0,o0) | w(k0,o1) | w(k1,o0) | w(k1,o1) | b_o0 b_o1 | xT_h0 | xT_h1]
Output [EPC, 128, 2, C] fp16 (o_half-partitioned), untransposed on host.
"""

import numpy as np

B, E, DIN, DOUT = 4096, 32, 256, 256
NCORES = 8
EPC = E // NCORES

TRACE = False
LAST_RESULT = None

_PROGRAM_CACHE = {}


def _build_program(C):
    import concourse.bass as bass
    import concourse.mybir as mybir
    import concourse.tile as tile
    from concourse import bacc

    f32 = mybir.dt.float32
    f16 = mybir.dt.float16

    W = 4 * 128 + 2 + 2 * C
    boff = 4 * 128
    xoff = boff + 2
    CK = 512                  # token chunk per PSUM group (f32 bank limit)

    nc = bacc.Bacc("TRN2", target_bir_lowering=False, debug=False,
                   enable_asserts=False)

    blk_d = nc.dram_tensor("blk", [EPC, 128, W], f16, kind="ExternalInput")
    bc_d = nc.dram_tensor("bcol", [128, EPC * 2], f32, kind="ExternalInput")
    out_d = nc.dram_tensor("out", [EPC, 128, 2, C], f16,
                           kind="ExternalOutput")

    with tile.TileContext(nc) as tc:
        with (
            tc.tile_pool(name="bin", bufs=4) as bpool,
            tc.tile_pool(name="oout", bufs=4) as opool,
            tc.tile_pool(name="psum", bufs=6, space=bass.MemorySpace.PSUM)
                as ppool,
        ):
            bct = bpool.tile([128, EPC * 2], f32, tag="bcol")
            nc.sync.dma_start(bct[:], bc_d.ap())
            blks = []
            for e in range(EPC):
                blk = bpool.tile([128, W], f16)
                eng = nc.sync if e % 2 == 0 else nc.scalar
                eng.dma_start(blk[:], blk_d.ap()[e])
                blks.append(blk)

            for e in range(EPC):
                blk = blks[e]
                ot = opool.tile([128, 2, C], f16)
                for oh in range(2):
                    for ck in range(0, C, CK):
                        cw = min(CK, C - ck)
                        ps = ppool.tile([128, CK], f32)
                        for k in range(2):
                            nc.tensor.matmul(
                                ps[:, :cw],
                                blk[:, (k * 2 + oh) * 128:
                                    (k * 2 + oh + 1) * 128],
                                blk[:, xoff + k * C + ck:
                                    xoff + k * C + ck + cw],
                                start=(k == 0), stop=(k == 1),
                            )
                        nc.vector.tensor_scalar_add(
                            ot[:, oh, ck:ck + cw], ps[:, :cw],
                            bct[:, e * 2 + oh:e * 2 + oh + 1])
                eng = nc.sync if e % 2 == 0 else nc.scalar
                eng.dma_start(out_d.ap()[e], ot[:])

    nc.compile()
    return nc


def _route(index):
    order = np.argsort(index, kind="stable")
    counts = np.bincount(index, minlength=E)
    offs = np.zeros(E + 1, np.int64)
    offs[1:] = np.cumsum(counts)
    C = max(64, int(-(-int(counts.max()) // 64) * 64))
    return order, counts, offs, C


def _pack_core(x16, w16, b16, order, offs, C, c):
    W = 4 * 128 + 2 + 2 * C
    boff = 4 * 128
    xoff = boff + 2
    blk = np.zeros((EPC, 128, W), np.float16)
    for sl in range(EPC):
        e = c * EPC + sl
        toks = order[offs[e]:offs[e + 1]]
        xT = x16[toks].T
        for k in range(2):
            for oh in range(2):
                blk[sl, :, (k * 2 + oh) * 128:(k * 2 + oh + 1) * 128] = \
                    w16[e, k * 128:(k + 1) * 128, oh * 128:(oh + 1) * 128]
        blk[sl, :, boff] = b16[e, 0:128]
        blk[sl, :, boff + 1] = b16[e, 128:256]
        blk[sl, :, xoff:xoff + xT.shape[1]] = xT[0:128]
        blk[sl, :, xoff + C:xoff + C + xT.shape[1]] = xT[128:256]
    return np.ascontiguousarray(blk)


def kernel(x, index, weight, bias):
    from concourse.bass_utils import run_bass_kernel_spmd

    global LAST_RESULT

    x = np.asarray(x, np.float32)
    index = np.asarray(index, np.int32)
    weight = np.asarray(weight, np.float32)
    bias = np.asarray(bias, np.float32)

    order, counts, offs, C = _route(index)

    if C not in _PROGRAM_CACHE:
        _PROGRAM_CACHE[C] = _build_program(C)
    nc = _PROGRAM_CACHE[C]

    x16 = x.astype(np.float16)
    w16 = weight.astype(np.float16)
    b16 = bias.astype(np.float16)
    in_maps = []
    for c in range(NCORES):
        bcol = bias[c * EPC:(c + 1) * EPC].reshape(EPC * 2, 128).T
        in_maps.append({
            "blk": _pack_core(x16, w16, b16, order, offs, C, c),
            "bcol": np.ascontiguousarray(bcol, np.float32),
        })

    kwargs = {}
    if TRACE:
        kwargs = dict(trace=True, trace_cores=list(range(NCORES)))
    res = run_bass_kernel_spmd(nc, in_maps, core_ids=list(range(NCORES)),
                               **kwargs)
    LAST_RESULT = res

    out = np.empty((B, DOUT), np.float32)
    for c in range(NCORES):
        oc = res.results[c]["out"]  # [EPC, 128, 2, C] fp16
        for sl in range(EPC):
            e = c * EPC + sl
            toks = order[offs[e]:offs[e + 1]]
            oe = oc[sl].transpose(2, 1, 0).reshape(C, DOUT)
            out[toks] = oe[:len(toks)].astype(np.float32)
    return out
